# revision 46
# baseline (speedup 1.0000x reference)
"""Trainium2 Bass kernel for nn_Encoder_inter: coif1 wavelet disentangle along
the node axis (dense banded 512x512 matrix, precomputed on host) followed by a
2-layer MLP (64->256->256) with ReLU, pointwise over (B, N, T).

Sharding: data-parallel over batch B=32 across 8 NeuronCores (4 batches each);
Linear weights and the wavelet matrix replicated.

v9: DMA-count diet + three-engine balance. Every dma_start costs ~600ns of
trigger time and ~115ns of end-of-kernel semaphore churn, so host layouts are
packed to make each transfer a single trigger: x as [BPC,128,4*TD] (1/batch),
KT as [128,4*N], W2T as [128,2*G], biases as one [128,4]. Output staged in
half-batch groups (12 t's) -> 2 DMAs per (batch,gc), final group split so the
tail transfer is small. Per t-pair iteration (1024 tokens):
  PE : 4 wavelet MMs (amortized quad) + 8 MLP2 MMs (free=512) + 4 MLP1 MMs
       (K=64, hw pairs them) -- K=128 block kept contiguous, 2 K-transitions
  ACT: 2 bias+Relu h-activations (128,1024) + y-copy (128,1024)/2
  DVE: 2 bias+relu out tensor_scalars (128,1024) psum->stg (2-bank reads)
MLP1 runs 2 iterations ahead of MLP2. PSUM: ph (yq/hps) 4 banks + po 4 banks.
"""
import os
import sys

for _p in ("/opt/trn_rl_repo", "/root/.axon_site/_ro/trn_rl_repo"):
    if os.path.isdir(_p) and _p not in sys.path:
        sys.path.insert(0, _p)

from contextlib import ExitStack

import numpy as np

import concourse.bass as bass
import concourse.tile as tile
from concourse import bacc, mybir
from concourse.bass_utils import run_bass_kernel_spmd

F32 = mybir.dt.float32
BF16 = mybir.dt.bfloat16

B, N, T, D, H, G = 32, 512, 24, 64, 256, 256
NCORES = 8
BPC = B // NCORES          # batches per core
TD = T * D                 # 1536
MCHUNK = N // 128          # 4
NTP = T // 2               # 12 t-pairs per batch
TGROUP = 12                # t's per output staging group (half batch)

# ---------------------------------------------------------------------------
# Host-side wavelet matrix (dwt -> 2*cD -> idwt along nodes == y = K @ x).
# ---------------------------------------------------------------------------
_L = 6
_DEC_LO = np.array(
    [-0.01565572813546454, -0.0727326195128539, 0.38486484686420286,
     0.8525720202122554, 0.3378976624578092, -0.0727326195128539],
    dtype=np.float64,
)
_DEC_HI = np.array(
    [0.0727326195128539, 0.3378976624578092, -0.8525720202122554,
     0.38486484686420286, 0.0727326195128539, -0.01565572813546454],
    dtype=np.float64,
)
_REC_LO = _DEC_LO[::-1].copy()
_REC_HI = _DEC_HI[::-1].copy()


def _dwt_last(x):
    n = x.shape[-1]
    ext = np.concatenate(
        [x[..., : _L - 1][..., ::-1], x, x[..., -(_L - 1):][..., ::-1]], axis=-1
    )
    out = (n + _L - 2) // 2
    cA = sum(_DEC_LO[j] * ext[..., _L - j: _L - j + 2 * out: 2] for j in range(_L))
    cD = sum(_DEC_HI[j] * ext[..., _L - j: _L - j + 2 * out: 2] for j in range(_L))
    return cA, cD


def _idwt_last(cA, cD, n):
    out = cA.shape[-1]
    up_shape = cA.shape[:-1] + (2 * out - 1,)
    upA = np.zeros(up_shape, cA.dtype)
    upA[..., ::2] = cA
    upD = np.zeros(up_shape, cD.dtype)
    upD[..., ::2] = cD
    pad = [(0, 0)] * (cA.ndim - 1) + [(_L - 1, _L - 1)]
    uA = np.pad(upA, pad)
    uD = np.pad(upD, pad)
    return sum(
        _REC_LO[j] * uA[..., 2 * _L - 3 - j: 2 * _L - 3 - j + n]
        + _REC_HI[j] * uD[..., 2 * _L - 3 - j: 2 * _L - 3 - j + n]
        for j in range(_L)
    )


def _wavelet_kt() -> np.ndarray:
    """K^T (m_in, n_out) so that (op(x))[n] = sum_m x[m] * KT[m, n]."""
    eye = np.eye(N, dtype=np.float64)
    cA, cD = _dwt_last(eye)
    kt = _idwt_last(cA, 2.0 * cD, N)
    return kt.astype(np.float32)


# ---------------------------------------------------------------------------
# Device kernel (SPMD, identical program on all 8 cores)
# ---------------------------------------------------------------------------
_NC_CACHE = None


def _build_nc():
    nc = bacc.Bacc("TRN2", target_bir_lowering=False, debug=False, num_devices=NCORES)
    # x packed [128, mc*TD] per batch so one DMA loads a whole batch
    x_d = nc.dram_tensor("x", [BPC, 128, MCHUNK * TD], BF16,
                         kind="ExternalInput").ap()
    kt_d = nc.dram_tensor("KT", [128, MCHUNK * N], BF16, kind="ExternalInput").ap()
    w1_d = nc.dram_tensor("W1T", [2 * D, H], BF16, kind="ExternalInput").ap()
    w2_d = nc.dram_tensor("W2T", [128, 2 * G], BF16, kind="ExternalInput").ap()
    # biases packed [128, 4]: cols = b1 chunk0, b1 chunk1, b2 chunk0, b2 chunk1
    bc_d = nc.dram_tensor("bc", [128, 4], F32, kind="ExternalInput").ap()
    # out[b, gc, g, t, n]; host transposes to (b, n, t, gc*128+g)
    out_d = nc.dram_tensor("out", [BPC, 2, 128, T, N], BF16, kind="ExternalOutput").ap()

    relu = mybir.ActivationFunctionType.Relu
    add_op = mybir.AluOpType.add
    max_op = mybir.AluOpType.max

    with tile.TileContext(nc) as tc, ExitStack() as ctx:
        consts = ctx.enter_context(tc.tile_pool(name="consts", bufs=1))
        xpool = ctx.enter_context(tc.tile_pool(name="xp", bufs=2))
        ypool = ctx.enter_context(tc.tile_pool(name="yp", bufs=3))
        hpool = ctx.enter_context(tc.tile_pool(name="hp", bufs=3))
        spool = ctx.enter_context(tc.tile_pool(name="sp", bufs=2))
        ph = ctx.enter_context(tc.tile_pool(name="ph", bufs=2, space="PSUM"))
        po = ctx.enter_context(tc.tile_pool(name="po", bufs=2, space="PSUM"))

        # --- replicated constants (gpsimd queue, parallel with x loads) ---
        kt_sb = consts.tile([128, MCHUNK * N], BF16, tag="kt", name="kt")
        nc.gpsimd.dma_start(out=kt_sb[:], in_=kt_d[:])
        w1_sb = consts.tile([2 * D, H], BF16, tag="w1", name="w1")
        nc.gpsimd.dma_start(out=w1_sb[:], in_=w1_d[:])
        w2_sb = consts.tile([128, 2 * G], BF16, tag="w2", name="w2")
        nc.gpsimd.dma_start(out=w2_sb[:], in_=w2_d[:])
        bc_sb = consts.tile([128, 4], F32, tag="bc", name="bc")
        nc.gpsimd.dma_start(out=bc_sb[:], in_=bc_d[:])

        iters = [(b, tp) for b in range(BPC) for tp in range(NTP)]
        x_sb = {}

        def load_x(b, split=False):
            xt = xpool.tile([128, MCHUNK * TD], BF16, tag="x", name=f"x{b}")
            if split:
                # head: t-pairs 0..1 (quad 0), then 2..3 (quad 2), then rest.
                # Measured best: the compute stream starting slightly after
                # the DMA stream keeps the loop free of DMA contention
                for lo, hi in ((0, 512), (512, 1024), (1024, TD)):
                    nc.sync.dma_start(
                        out=xt[:].rearrange("p (m c) -> p m c",
                                            m=MCHUNK)[:, :, lo:hi],
                        in_=x_d[b].rearrange("p (m c) -> p m c",
                                             m=MCHUNK)[:, :, lo:hi],
                    )
            else:
                nc.gpsimd.dma_start(out=xt[:], in_=x_d[b])
            x_sb[b] = xt

        def wavelet_quad(e, on_dve=False):
            """8 banded MMs (2 iterations = 2 t-pairs) into a 2-bank ph tile.
            The (128,1024) ACT y-copy is emitted separately (late in the
            iteration) so it never delays the h-relus in ACT's queue."""
            yq = ph.tile([128, 2 * N], F32, tag="hps", name="yq")
            for half in range(2):
                b, tp = iters[e + half]
                t0 = 2 * tp
                for mc in range(MCHUNK):
                    if mc == 0:
                        lo, hi = 0, 132
                    else:
                        lo, hi = 128 * mc - 4, min(N, 128 * mc + 132)
                    nc.tensor.matmul(
                        yq[:, half * N + lo:half * N + hi],
                        lhsT=x_sb[b][:, mc * TD + t0 * D:mc * TD + (t0 + 2) * D],
                        rhs=kt_sb[:, mc * N + lo:mc * N + hi],
                        start=(mc == 0),
                        stop=(mc == MCHUNK - 1),
                        skip_group_check=True,
                    )
            y_sb = ypool.tile([128, 2 * N], BF16, tag="yt", name="y_sb")
            if on_dve:
                nc.vector.tensor_scalar_add(y_sb[:], yq[:], 0.0)
            else:
                nc.scalar.copy(y_sb[:], yq[:])
            return y_sb

        def mlp1(y_sb, half):
            """4 K=64 MMs (hw auto-pairs same-K neighbours) + 2 bias+Relu ACT
            activations (128,1024) -> bf16 h."""
            h_sb = []
            for hc in range(2):
                hps = ph.tile([128, 2 * N], F32, tag="hps", name="hps")
                for ti in range(2):
                    nc.tensor.matmul(
                        hps[:, ti * N:(ti + 1) * N],
                        lhsT=w1_sb[ti * D:(ti + 1) * D,
                                   hc * 128:(hc + 1) * 128],
                        rhs=y_sb[ti * D:(ti + 1) * D,
                                 half * N:(half + 1) * N],
                        start=True,
                        stop=True,
                        skip_group_check=True,
                    )
                hs = hpool.tile([128, 2 * N], BF16, tag=f"h1_{hc}",
                                name=f"h1_{hc}")
                nc.scalar.activation(hs[:], hps[:], relu,
                                     bias=bc_sb[:, hc:hc + 1])
                h_sb.append(hs)
            return h_sb

        def mlp2_gc(h_sb, stg, gc, slot0, on_act=False):
            """One gc of MLP2: 4 MMs (free=512, one PSUM bank each) into a
            2-bank po tile, drained by one (128,1024) relu. Normally on DVE;
            the final group's gc1 relus go to ACT (idle at the end) so the
            two last relus run in parallel and the tail DMA fires earlier."""
            ops = po.tile([128, 2 * N], F32, tag="ops", name="ops")
            for hc in range(2):
                for ti in range(2):
                    nc.tensor.matmul(
                        ops[:, ti * N:(ti + 1) * N],
                        lhsT=w2_sb[:, hc * G + gc * 128:hc * G + (gc + 1) * 128],
                        rhs=h_sb[hc][:, ti * N:(ti + 1) * N],
                        start=(hc == 0),
                        stop=(hc == 1),
                        skip_group_check=True,
                    )
            if on_act:
                nc.scalar.activation(
                    stg[gc][:, slot0 * N:(slot0 + 2) * N],
                    ops[:], relu, bias=bc_sb[:, 2 + gc:3 + gc],
                )
            else:
                nc.vector.tensor_scalar(
                    stg[gc][:, slot0 * N:(slot0 + 2) * N],
                    ops[:], bc_sb[:, 2 + gc:3 + gc], 0.0, add_op, max_op,
                )

        # HAM warmup: dummy matmuls so the PE clock gate opens while the
        # first x/weight DMAs are still in flight.
        scratch = consts.tile([128, N], BF16, tag="scratch", name="scratch")
        # memset on DVE: its queue is empty at start, while gpsimd's holds
        # the const DMA triggers (gpsimd.memset delays the warmup ~6us)
        nc.vector.memset(scratch[:], 0.0)
        wps = po.tile([128, 2 * N], F32, tag="ops", name="warm")
        for wi in range(12):
            nc.tensor.matmul(
                wps[:, 0:N],
                lhsT=scratch[:, 0:128],
                rhs=scratch[:],
                start=(wi == 0),
                stop=(wi == 11),
                skip_group_check=True,
            )

        # prologue: quad wavelets 4 iterations ahead, MLP1 two ahead
        load_x(0, split=True)
        y0 = wavelet_quad(0)
        # keep the PE busy while quad 2's x chunk is still in flight
        wps2 = po.tile([128, 2 * N], F32, tag="ops", name="warm2")
        for wi in range(2):
            nc.tensor.matmul(
                wps2[:, 0:N],
                lhsT=scratch[:, 0:128],
                rhs=scratch[:],
                start=(wi == 0),
                stop=(wi == 1),
                skip_group_check=True,
            )
        # y2's copy on DVE (idle in the prologue) so it runs in parallel
        # with y0's on ACT and mlp1's hps slots free ~1us earlier
        y2 = wavelet_quad(2, on_dve=True)
        h_queue = [mlp1(y0, 0)]
        h_queue.append(mlp1(y0, 1))
        y_quads = [y0, y2]

        stg = None
        for it, (b, tp) in enumerate(iters):
            t0 = 2 * tp
            grp = t0 // TGROUP
            slot0 = t0 % TGROUP
            final_grp = (b == BPC - 1 and grp == 1)
            if slot0 == 0:
                stg = [
                    spool.tile([128, TGROUP * N], BF16, tag=f"stg{gc}",
                               name=f"stg{gc}")
                    for gc in range(2)
                ]
            if tp == 6 and b + 1 < BPC:
                load_x(b + 1)
            # all K=128 matmuls (both MLP2 gc's + quad between them) before
            # the K=64 MLP1 block: each K-width transition costs a PE reconfig
            # bubble. The quad sits after gc0 so its yq slot WAR (on an ACT
            # h-relu completion) has ~1us more headroom than at iter start.
            mlp2_gc(h_queue[0], stg, 0, slot0)        # back half of k (gc0)
            if slot0 == TGROUP - 2 and not final_grp:
                nc.sync.dma_start(
                    out=out_d[b, 0, :, grp * TGROUP:(grp + 1) * TGROUP, :],
                    in_=stg[0][:].rearrange("p (t n) -> p t n", t=TGROUP),
                )
            if it % 2 == 0 and it + 5 < len(iters):
                y_quads.append(wavelet_quad(it + 4))  # quad for k+4, k+5
            mlp2_gc(h_queue[0], stg, 1, slot0,
                    on_act=final_grp)                 # back half of k (gc1)
            if slot0 == TGROUP - 2 and not final_grp:
                nc.sync.dma_start(
                    out=out_d[b, 1, :, grp * TGROUP:(grp + 1) * TGROUP, :],
                    in_=stg[1][:].rearrange("p (t n) -> p t n", t=TGROUP),
                )
            if it + 2 < len(iters):
                nx = it + 2
                if nx % 2 == 0:
                    y_quads.pop(0)
                h_queue.append(mlp1(y_quads[0], nx % 2))  # front of k+2
            if final_grp:
                # final group: ship 10 t's early, last 2 t's at the end so the
                # tail DMA is small
                if slot0 == 8:
                    for gc in range(2):
                        nc.sync.dma_start(
                            out=out_d[b, gc, :,
                                      grp * TGROUP:grp * TGROUP + 10, :],
                            in_=stg[gc][:, 0:10 * N].rearrange(
                                "p (t n) -> p t n", t=10),
                        )
                elif slot0 == TGROUP - 2:
                    for gc in range(2):
                        nc.sync.dma_start(
                            out=out_d[b, gc, :,
                                      grp * TGROUP + 10:grp * TGROUP + 12, :],
                            in_=stg[gc][:, 10 * N:12 * N].rearrange(
                                "p (t n) -> p t n", t=2),
                        )
            h_queue.pop(0)
    nc.compile()
    return nc


def _get_nc():
    global _NC_CACHE
    if _NC_CACHE is None:
        _NC_CACHE = _build_nc()
    return _NC_CACHE


def _make_in_maps(x, W1, b1, W2, b2):
    import ml_dtypes
    bf = ml_dtypes.bfloat16
    x = np.ascontiguousarray(np.asarray(x, dtype=np.float32))
    W1 = np.asarray(W1, dtype=np.float32)
    b1 = np.asarray(b1, dtype=np.float32)
    W2 = np.asarray(W2, dtype=np.float32)
    b2 = np.asarray(b2, dtype=np.float32)

    kt = np.ascontiguousarray(
        _wavelet_kt().reshape(MCHUNK, 128, N).transpose(1, 0, 2)
        .reshape(128, MCHUNK * N)
    ).astype(bf)
    w1t = np.ascontiguousarray(np.concatenate([W1.T, W1.T], axis=0)).astype(bf)
    # W2T packed [128, 2*G]: cols hc*G+g hold W2[g, hc*128+p]
    w2t = np.ascontiguousarray(
        W2.T.reshape(2, 128, G).transpose(1, 0, 2).reshape(128, 2 * G)
    ).astype(bf)
    bc = np.ascontiguousarray(
        np.stack([b1[0:128], b1[128:256], b2[0:128], b2[128:256]], axis=1)
    ).astype(np.float32)

    in_maps = []
    for c in range(NCORES):
        xc = x[c * BPC:(c + 1) * BPC].reshape(BPC, MCHUNK, 128, TD)
        xc = np.ascontiguousarray(
            xc.transpose(0, 2, 1, 3).reshape(BPC, 128, MCHUNK * TD)
        ).astype(bf)
        in_maps.append({"x": xc, "KT": kt, "W1T": w1t, "W2T": w2t, "bc": bc})
    return in_maps


def kernel(x, W1, b1, W2, b2):
    nc = _get_nc()
    in_maps = _make_in_maps(x, W1, b1, W2, b2)
    res = run_bass_kernel_spmd(nc, in_maps, list(range(NCORES)))
    # device out: [BPC, 2, 128, T, N] per core -> (B, N, T, G)
    out = np.concatenate(
        [res.results[c]["out"].astype(np.float32) for c in range(NCORES)], axis=0
    )
    out = out.transpose(0, 4, 3, 1, 2).reshape(B, N, T, G)
    return np.ascontiguousarray(out)


# revision 47
# speedup vs baseline: 1.0204x; 1.0204x over previous
"""Trainium2 Bass kernel for nn_Encoder_inter: coif1 wavelet disentangle along
the node axis (dense banded 512x512 matrix, precomputed on host) followed by a
2-layer MLP (64->256->256) with ReLU, pointwise over (B, N, T).

Sharding: data-parallel over batch B=32 across 8 NeuronCores (4 batches each);
Linear weights and the wavelet matrix replicated.

v9: DMA-count diet + three-engine balance. Every dma_start costs ~600ns of
trigger time and ~115ns of end-of-kernel semaphore churn, so host layouts are
packed to make each transfer a single trigger: x as [BPC,128,4*TD] (1/batch),
KT as [128,4*N], W2T as [128,2*G], biases as one [128,4]. Output staged in
half-batch groups (12 t's) -> 2 DMAs per (batch,gc), final group split so the
tail transfer is small. Per t-pair iteration (1024 tokens):
  PE : 4 wavelet MMs (amortized quad) + 8 MLP2 MMs (free=512) + 4 MLP1 MMs
       (K=64, hw pairs them) -- K=128 block kept contiguous, 2 K-transitions
  ACT: 2 bias+Relu h-activations (128,1024) + y-copy (128,1024)/2
  DVE: 2 bias+relu out tensor_scalars (128,1024) psum->stg (2-bank reads)
MLP1 runs 2 iterations ahead of MLP2. PSUM: ph (yq/hps) 4 banks + po 4 banks.
"""
import os
import sys

for _p in ("/opt/trn_rl_repo", "/root/.axon_site/_ro/trn_rl_repo"):
    if os.path.isdir(_p) and _p not in sys.path:
        sys.path.insert(0, _p)

from contextlib import ExitStack

import numpy as np

import concourse.bass as bass
import concourse.tile as tile
from concourse import bacc, mybir
from concourse.bass_utils import run_bass_kernel_spmd

F32 = mybir.dt.float32
BF16 = mybir.dt.bfloat16

B, N, T, D, H, G = 32, 512, 24, 64, 256, 256
NCORES = 8
BPC = B // NCORES          # batches per core
TD = T * D                 # 1536
MCHUNK = N // 128          # 4
NTP = T // 2               # 12 t-pairs per batch
TGROUP = 12                # t's per output staging group (half batch)

# ---------------------------------------------------------------------------
# Host-side wavelet matrix (dwt -> 2*cD -> idwt along nodes == y = K @ x).
# ---------------------------------------------------------------------------
_L = 6
_DEC_LO = np.array(
    [-0.01565572813546454, -0.0727326195128539, 0.38486484686420286,
     0.8525720202122554, 0.3378976624578092, -0.0727326195128539],
    dtype=np.float64,
)
_DEC_HI = np.array(
    [0.0727326195128539, 0.3378976624578092, -0.8525720202122554,
     0.38486484686420286, 0.0727326195128539, -0.01565572813546454],
    dtype=np.float64,
)
_REC_LO = _DEC_LO[::-1].copy()
_REC_HI = _DEC_HI[::-1].copy()


def _dwt_last(x):
    n = x.shape[-1]
    ext = np.concatenate(
        [x[..., : _L - 1][..., ::-1], x, x[..., -(_L - 1):][..., ::-1]], axis=-1
    )
    out = (n + _L - 2) // 2
    cA = sum(_DEC_LO[j] * ext[..., _L - j: _L - j + 2 * out: 2] for j in range(_L))
    cD = sum(_DEC_HI[j] * ext[..., _L - j: _L - j + 2 * out: 2] for j in range(_L))
    return cA, cD


def _idwt_last(cA, cD, n):
    out = cA.shape[-1]
    up_shape = cA.shape[:-1] + (2 * out - 1,)
    upA = np.zeros(up_shape, cA.dtype)
    upA[..., ::2] = cA
    upD = np.zeros(up_shape, cD.dtype)
    upD[..., ::2] = cD
    pad = [(0, 0)] * (cA.ndim - 1) + [(_L - 1, _L - 1)]
    uA = np.pad(upA, pad)
    uD = np.pad(upD, pad)
    return sum(
        _REC_LO[j] * uA[..., 2 * _L - 3 - j: 2 * _L - 3 - j + n]
        + _REC_HI[j] * uD[..., 2 * _L - 3 - j: 2 * _L - 3 - j + n]
        for j in range(_L)
    )


def _wavelet_kt() -> np.ndarray:
    """K^T (m_in, n_out) so that (op(x))[n] = sum_m x[m] * KT[m, n]."""
    eye = np.eye(N, dtype=np.float64)
    cA, cD = _dwt_last(eye)
    kt = _idwt_last(cA, 2.0 * cD, N)
    return kt.astype(np.float32)


# ---------------------------------------------------------------------------
# Device kernel (SPMD, identical program on all 8 cores)
# ---------------------------------------------------------------------------
_NC_CACHE = None


def _build_nc():
    nc = bacc.Bacc("TRN2", target_bir_lowering=False, debug=False, num_devices=NCORES)
    # x packed [128, mc*TD] per batch so one DMA loads a whole batch
    x_d = nc.dram_tensor("x", [BPC, 128, MCHUNK * TD], BF16,
                         kind="ExternalInput").ap()
    kt_d = nc.dram_tensor("KT", [128, MCHUNK * N], BF16, kind="ExternalInput").ap()
    w1_d = nc.dram_tensor("W1T", [2 * D, H], BF16, kind="ExternalInput").ap()
    w2_d = nc.dram_tensor("W2T", [128, 2 * G], BF16, kind="ExternalInput").ap()
    # biases packed [128, 4]: cols = b1 chunk0, b1 chunk1, b2 chunk0, b2 chunk1
    bc_d = nc.dram_tensor("bc", [128, 4], F32, kind="ExternalInput").ap()
    # out[b, gc, g, t, n]; host transposes to (b, n, t, gc*128+g)
    out_d = nc.dram_tensor("out", [BPC, 2, 128, T, N], BF16, kind="ExternalOutput").ap()

    relu = mybir.ActivationFunctionType.Relu
    add_op = mybir.AluOpType.add
    max_op = mybir.AluOpType.max

    with tile.TileContext(nc) as tc, ExitStack() as ctx:
        consts = ctx.enter_context(tc.tile_pool(name="consts", bufs=1))
        xpool = ctx.enter_context(tc.tile_pool(name="xp", bufs=2))
        ypool = ctx.enter_context(tc.tile_pool(name="yp", bufs=4))
        hpool = ctx.enter_context(tc.tile_pool(name="hp", bufs=4))
        spool = ctx.enter_context(tc.tile_pool(name="sp", bufs=2))
        ph = ctx.enter_context(tc.tile_pool(name="ph", bufs=2, space="PSUM"))
        po = ctx.enter_context(tc.tile_pool(name="po", bufs=2, space="PSUM"))

        # --- replicated constants (gpsimd queue, parallel with x loads) ---
        kt_sb = consts.tile([128, MCHUNK * N], BF16, tag="kt", name="kt")
        nc.gpsimd.dma_start(out=kt_sb[:], in_=kt_d[:])
        w1_sb = consts.tile([2 * D, H], BF16, tag="w1", name="w1")
        nc.gpsimd.dma_start(out=w1_sb[:], in_=w1_d[:])
        w2_sb = consts.tile([128, 2 * G], BF16, tag="w2", name="w2")
        nc.gpsimd.dma_start(out=w2_sb[:], in_=w2_d[:])
        bc_sb = consts.tile([128, 4], F32, tag="bc", name="bc")
        nc.gpsimd.dma_start(out=bc_sb[:], in_=bc_d[:])

        iters = [(b, tp) for b in range(BPC) for tp in range(NTP)]
        x_sb = {}

        def load_x(b, split=False):
            xt = xpool.tile([128, MCHUNK * TD], BF16, tag="x", name=f"x{b}")
            if split:
                # head: t-pairs 0..1 (quad 0), then 2..3 (quad 2), then rest.
                # Measured best: the compute stream starting slightly after
                # the DMA stream keeps the loop free of DMA contention
                for lo, hi in ((0, 512), (512, 1024), (1024, TD)):
                    nc.sync.dma_start(
                        out=xt[:].rearrange("p (m c) -> p m c",
                                            m=MCHUNK)[:, :, lo:hi],
                        in_=x_d[b].rearrange("p (m c) -> p m c",
                                             m=MCHUNK)[:, :, lo:hi],
                    )
            else:
                nc.gpsimd.dma_start(out=xt[:], in_=x_d[b])
            x_sb[b] = xt

        def wavelet_quad(e, on_dve=False):
            """8 banded MMs (2 iterations = 2 t-pairs) into a 2-bank ph tile.
            The (128,1024) ACT y-copy is emitted separately (late in the
            iteration) so it never delays the h-relus in ACT's queue."""
            yq = ph.tile([128, 2 * N], F32, tag="hps", name="yq")
            for half in range(2):
                b, tp = iters[e + half]
                t0 = 2 * tp
                for mc in range(MCHUNK):
                    if mc == 0:
                        lo, hi = 0, 132
                    else:
                        lo, hi = 128 * mc - 4, min(N, 128 * mc + 132)
                    nc.tensor.matmul(
                        yq[:, half * N + lo:half * N + hi],
                        lhsT=x_sb[b][:, mc * TD + t0 * D:mc * TD + (t0 + 2) * D],
                        rhs=kt_sb[:, mc * N + lo:mc * N + hi],
                        start=(mc == 0),
                        stop=(mc == MCHUNK - 1),
                        skip_group_check=True,
                    )
            y_sb = ypool.tile([128, 2 * N], BF16, tag="yt", name="y_sb")
            if on_dve:
                nc.vector.tensor_scalar_add(y_sb[:], yq[:], 0.0)
            else:
                nc.scalar.copy(y_sb[:], yq[:])
            return y_sb

        def mlp1(y_sb, half):
            """4 K=64 MMs (hw auto-pairs same-K neighbours) + 2 bias+Relu ACT
            activations (128,1024) -> bf16 h."""
            h_sb = []
            for hc in range(2):
                hps = ph.tile([128, 2 * N], F32, tag="hps", name="hps")
                for ti in range(2):
                    nc.tensor.matmul(
                        hps[:, ti * N:(ti + 1) * N],
                        lhsT=w1_sb[ti * D:(ti + 1) * D,
                                   hc * 128:(hc + 1) * 128],
                        rhs=y_sb[ti * D:(ti + 1) * D,
                                 half * N:(half + 1) * N],
                        start=True,
                        stop=True,
                        skip_group_check=True,
                    )
                hs = hpool.tile([128, 2 * N], BF16, tag=f"h1_{hc}",
                                name=f"h1_{hc}")
                nc.scalar.activation(hs[:], hps[:], relu,
                                     bias=bc_sb[:, hc:hc + 1])
                h_sb.append(hs)
            return h_sb

        def mlp2_gc(h_sb, stg, gc, slot0, on_act=False):
            """One gc of MLP2: 4 MMs (free=512, one PSUM bank each) into a
            2-bank po tile, drained by one (128,1024) relu. Normally on DVE;
            the final group's gc1 relus go to ACT (idle at the end) so the
            two last relus run in parallel and the tail DMA fires earlier."""
            ops = po.tile([128, 2 * N], F32, tag="ops", name="ops")
            for hc in range(2):
                for ti in range(2):
                    nc.tensor.matmul(
                        ops[:, ti * N:(ti + 1) * N],
                        lhsT=w2_sb[:, hc * G + gc * 128:hc * G + (gc + 1) * 128],
                        rhs=h_sb[hc][:, ti * N:(ti + 1) * N],
                        start=(hc == 0),
                        stop=(hc == 1),
                        skip_group_check=True,
                    )
            if on_act:
                nc.scalar.activation(
                    stg[gc][:, slot0 * N:(slot0 + 2) * N],
                    ops[:], relu, bias=bc_sb[:, 2 + gc:3 + gc],
                )
            else:
                nc.vector.tensor_scalar(
                    stg[gc][:, slot0 * N:(slot0 + 2) * N],
                    ops[:], bc_sb[:, 2 + gc:3 + gc], 0.0, add_op, max_op,
                )

        # HAM warmup: dummy matmuls so the PE clock gate opens while the
        # first x/weight DMAs are still in flight.
        scratch = consts.tile([128, N], BF16, tag="scratch", name="scratch")
        # memset on DVE: its queue is empty at start, while gpsimd's holds
        # the const DMA triggers (gpsimd.memset delays the warmup ~6us)
        nc.vector.memset(scratch[:], 0.0)
        wps = po.tile([128, 2 * N], F32, tag="ops", name="warm")
        for wi in range(12):
            nc.tensor.matmul(
                wps[:, 0:N],
                lhsT=scratch[:, 0:128],
                rhs=scratch[:],
                start=(wi == 0),
                stop=(wi == 11),
                skip_group_check=True,
            )

        # prologue: quad wavelets 4 iterations ahead, MLP1 two ahead
        load_x(0, split=True)
        y0 = wavelet_quad(0)
        # keep the PE busy while quad 2's x chunk is still in flight
        wps2 = po.tile([128, 2 * N], F32, tag="ops", name="warm2")
        for wi in range(2):
            nc.tensor.matmul(
                wps2[:, 0:N],
                lhsT=scratch[:, 0:128],
                rhs=scratch[:],
                start=(wi == 0),
                stop=(wi == 1),
                skip_group_check=True,
            )
        # y2's copy on DVE (idle in the prologue) so it runs in parallel
        # with y0's on ACT and mlp1's hps slots free ~1us earlier
        y2 = wavelet_quad(2, on_dve=True)
        h_queue = [mlp1(y0, 0)]
        h_queue.append(mlp1(y0, 1))
        y_quads = [y0, y2]

        stg = None
        for it, (b, tp) in enumerate(iters):
            t0 = 2 * tp
            grp = t0 // TGROUP
            slot0 = t0 % TGROUP
            final_grp = (b == BPC - 1 and grp == 1)
            if slot0 == 0:
                stg = [
                    spool.tile([128, TGROUP * N], BF16, tag=f"stg{gc}",
                               name=f"stg{gc}")
                    for gc in range(2)
                ]
            if tp == 6 and b + 1 < BPC:
                load_x(b + 1)
            # all K=128 matmuls (both MLP2 gc's + quad between them) before
            # the K=64 MLP1 block: each K-width transition costs a PE reconfig
            # bubble. The quad sits after gc0 so its yq slot WAR (on an ACT
            # h-relu completion) has ~1us more headroom than at iter start.
            mlp2_gc(h_queue[0], stg, 0, slot0)        # back half of k (gc0)
            if slot0 == TGROUP - 2 and not final_grp:
                nc.sync.dma_start(
                    out=out_d[b, 0, :, grp * TGROUP:(grp + 1) * TGROUP, :],
                    in_=stg[0][:].rearrange("p (t n) -> p t n", t=TGROUP),
                )
            if it % 2 == 0 and it + 5 < len(iters):
                y_quads.append(wavelet_quad(it + 4))  # quad for k+4, k+5
            mlp2_gc(h_queue[0], stg, 1, slot0,
                    on_act=final_grp)                 # back half of k (gc1)
            if slot0 == TGROUP - 2 and not final_grp:
                nc.sync.dma_start(
                    out=out_d[b, 1, :, grp * TGROUP:(grp + 1) * TGROUP, :],
                    in_=stg[1][:].rearrange("p (t n) -> p t n", t=TGROUP),
                )
            if it + 2 < len(iters):
                nx = it + 2
                if nx % 2 == 0:
                    y_quads.pop(0)
                h_queue.append(mlp1(y_quads[0], nx % 2))  # front of k+2
            if final_grp:
                # final group: ship 10 t's early, last 2 t's at the end so the
                # tail DMA is small
                if slot0 == 8:
                    for gc in range(2):
                        nc.sync.dma_start(
                            out=out_d[b, gc, :,
                                      grp * TGROUP:grp * TGROUP + 10, :],
                            in_=stg[gc][:, 0:10 * N].rearrange(
                                "p (t n) -> p t n", t=10),
                        )
                elif slot0 == TGROUP - 2:
                    for gc in range(2):
                        nc.sync.dma_start(
                            out=out_d[b, gc, :,
                                      grp * TGROUP + 10:grp * TGROUP + 12, :],
                            in_=stg[gc][:, 10 * N:12 * N].rearrange(
                                "p (t n) -> p t n", t=2),
                        )
            h_queue.pop(0)
    nc.compile()
    return nc


def _get_nc():
    global _NC_CACHE
    if _NC_CACHE is None:
        _NC_CACHE = _build_nc()
    return _NC_CACHE


def _make_in_maps(x, W1, b1, W2, b2):
    import ml_dtypes
    bf = ml_dtypes.bfloat16
    x = np.ascontiguousarray(np.asarray(x, dtype=np.float32))
    W1 = np.asarray(W1, dtype=np.float32)
    b1 = np.asarray(b1, dtype=np.float32)
    W2 = np.asarray(W2, dtype=np.float32)
    b2 = np.asarray(b2, dtype=np.float32)

    kt = np.ascontiguousarray(
        _wavelet_kt().reshape(MCHUNK, 128, N).transpose(1, 0, 2)
        .reshape(128, MCHUNK * N)
    ).astype(bf)
    w1t = np.ascontiguousarray(np.concatenate([W1.T, W1.T], axis=0)).astype(bf)
    # W2T packed [128, 2*G]: cols hc*G+g hold W2[g, hc*128+p]
    w2t = np.ascontiguousarray(
        W2.T.reshape(2, 128, G).transpose(1, 0, 2).reshape(128, 2 * G)
    ).astype(bf)
    bc = np.ascontiguousarray(
        np.stack([b1[0:128], b1[128:256], b2[0:128], b2[128:256]], axis=1)
    ).astype(np.float32)

    in_maps = []
    for c in range(NCORES):
        xc = x[c * BPC:(c + 1) * BPC].reshape(BPC, MCHUNK, 128, TD)
        xc = np.ascontiguousarray(
            xc.transpose(0, 2, 1, 3).reshape(BPC, 128, MCHUNK * TD)
        ).astype(bf)
        in_maps.append({"x": xc, "KT": kt, "W1T": w1t, "W2T": w2t, "bc": bc})
    return in_maps


def kernel(x, W1, b1, W2, b2):
    nc = _get_nc()
    in_maps = _make_in_maps(x, W1, b1, W2, b2)
    res = run_bass_kernel_spmd(nc, in_maps, list(range(NCORES)))
    # device out: [BPC, 2, 128, T, N] per core -> (B, N, T, G)
    out = np.concatenate(
        [res.results[c]["out"].astype(np.float32) for c in range(NCORES)], axis=0
    )
    out = out.transpose(0, 4, 3, 1, 2).reshape(B, N, T, G)
    return np.ascontiguousarray(out)
